# revision 22
# baseline (speedup 1.0000x reference)
"""nn_KNN Trainium2 kernel: sequential per-node neighbor-mean scan as one GEMM.

The reference's scan is a composition of per-column linear updates, so
out = x0 @ M for a precomputable M. Folding the initial mask-fill into M
(zeroing the unknown rows -> M', bias r), known columns pass through
exactly and only the 256 unknown columns need compute:

  out[:, known]   = input[:, known]                     (host pass-through)
  out[:, unknown] = input[:, known] @ Vk + r            (r added on host)

Sharding: batch b -> core b (data parallel, no collectives).

Device-side scheme (per core):
- x and Vk are quantized to fp8-e4m3 and the GEMM runs in DoubleRow perf
  mode (256-deep contraction per pass, 3 passes for K=768), which is the
  PE-minimal schedule: the PE streams one moving column per cycle
  regardless of dtype, so time = out-columns x passes. Accumulation is
  fp32 in PSUM; output is bf16. Measured end-to-end rel err 1.18e-2
  against the fp32 reference (budget 2e-2), bit-reproducible and equal
  to the host emulation of the same quantization.
- x is packed per time-chunk as contiguous [128, 6*chunk] blocks so each
  chunk is ONE dma with 128 fat descriptors; in fp8 the whole slice is
  SBUF-resident so every chunk gets its own tile (no buffer recycling
  stalls). Chunks ramp 256->1024: the DGE round-robins across queued
  transfers, so the first chunk must be small for the first matmul to
  start early, and deep issue backlogs are avoided by loading one chunk
  ahead of use.
- Inputs are issued on the sync-engine DGE ring in consumption order
  (Vk first); output quarter-DMAs live on the scalar-engine ring; psum
  drains alternate vector/scalar engines.
"""

import sys

import numpy as np

try:
    import concourse.bass  # noqa: F401
except ImportError:  # pragma: no cover
    sys.path.insert(0, "/opt/trn_rl_repo")

import concourse.bacc as bacc_mod
import concourse.mybir as mybir
from concourse.bass_utils import run_bass_kernel_spmd
from concourse.tile import TileContext

B, T, N, NS = 8, 4096, 1024, 256
NK = N - NS
P = 128
JC = NK // P  # 6 contraction tiles
KP = JC // 2  # 3 DoubleRow contraction pairs
SB = NS // P  # 2 output-partition tiles
MMF = 512  # psum bank free size (fp32)

# time-chunk schedule: every chunk a multiple of 512 so each psum tile is
# full-width (48 matmuls, the PE minimum); small tail chunks shorten the
# shutdown (last output store is only 512 cols)
CHUNKS = (512, 512, 1024, 1024, 512, 512)
# output groups (out-DMA granularity): chunk indices per group
OUTG = ((0, 1), (2,), (3,), (4,), (5,))
# PE warm-up matmuls on a memset tile while waiting for the first loads:
# ramps the PE clock out of its low p-state so real matmuls run at full
# rate immediately
WARMUP_MM = 16

BF16 = mybir.dt.bfloat16
FP8 = mybir.dt.float8e4
NP_BF16 = mybir.dt.np(mybir.dt.bfloat16)
NP_FP8 = mybir.dt.np(mybir.dt.float8e4)


def _build_kernel():
    nc = bacc_mod.Bacc("TRN2", target_bir_lowering=False, name="knn_fp8")
    f32 = mybir.dt.float32
    xC = nc.dram_tensor("xC", [P, JC * T], FP8, kind="ExternalInput")
    VkP = nc.dram_tensor("VkP", [P, JC * NS], FP8, kind="ExternalInput")
    outS = nc.dram_tensor("outS", [NS, T], BF16, kind="ExternalOutput")

    assert sum(CHUNKS) == T
    DR = mybir.MatmulPerfMode.DoubleRow

    with TileContext(nc) as tc:
        with (
            tc.tile_pool(name="consts", bufs=1) as cpool,
            tc.tile_pool(name="xt", bufs=1) as xpool,
            tc.tile_pool(name="outp", bufs=2) as opool,
            tc.tile_pool(name="ps", bufs=2, space="PSUM") as pspool,
        ):
            vk_sb = cpool.tile([P, JC * NS], FP8, tag="vk")
            nc.sync.dma_start(out=vk_sb, in_=VkP[:, :])
            vk_v = vk_sb.rearrange("p (a d s) -> p a d s", a=KP, d=2)
            vks = [vk_v[:, a] for a in range(KP)]

            # warm-up: no DMA dependency, so these run while inputs load
            warm = cpool.tile([P, 2 * MMF], FP8, tag="warm")
            nc.vector.memset(warm, 1.0)
            wv = warm.rearrange("p (d s) -> p d s", d=2)
            wps = pspool.tile([P, MMF], f32, tag="ps0_0", name="wps")
            for _ in range(WARMUP_MM):
                nc.tensor.matmul(
                    wps, lhsT=wv[:, :, :P], rhs=wv[:, :, :],
                    start=True, stop=True,
                    perf_mode=mybir.MatmulPerfMode.DoubleRow,
                )

            offs = [sum(CHUNKS[:i]) for i in range(len(CHUNKS))]

            def load_chunk(c):
                L = CHUNKS[c]
                xall = xpool.tile([P, JC * L], FP8, tag=f"xs{c}",
                                  name=f"xs{c}")
                nc.sync.dma_start(
                    out=xall,
                    in_=xC[:, JC * offs[c]:JC * (offs[c] + L)])
                return xall.rearrange("p (a d l) -> p a d l", a=KP, d=2)

            xvs = {0: load_chunk(0)}

            goff = [offs[g[0]] for g in OUTG]
            glen = [sum(CHUNKS[c] for c in g) for g in OUTG]
            ots = {}
            t0 = 0
            for c, L in enumerate(CHUNKS):
                if c + 1 < len(CHUNKS):
                    xvs[c + 1] = load_chunk(c + 1)
                xv = xvs[c]
                q = next(qi for qi, cs in enumerate(OUTG) if c in cs)
                qoff = t0 - goff[q]
                MT = (L + MMF - 1) // MMF
                for sb in range(SB):
                    if c == OUTG[q][0]:
                        ots[sb] = opool.tile([P, glen[q]], BF16,
                                             tag=f"og{q}_{sb}",
                                             name=f"og{q}_{sb}")
                    for m in range(MT):
                        w = min(MMF, L - m * MMF)
                        ps = pspool.tile([P, MMF], f32, tag=f"ps{sb}_{m}",
                                         name=f"ps{sb}_{m}")
                        for a in range(KP):
                            nc.tensor.matmul(
                                ps[:, :w],
                                lhsT=vks[a][:, :, sb * P:(sb + 1) * P],
                                rhs=xv[:, a, :, m * MMF:m * MMF + w],
                                start=(a == 0), stop=(a == KP - 1),
                                perf_mode=DR,
                            )
                        dst = ots[sb][:, qoff + m * MMF:qoff + m * MMF + w]
                        if sb == 0:
                            nc.vector.tensor_copy(out=dst, in_=ps[:, :w])
                        else:
                            nc.scalar.copy(out=dst, in_=ps[:, :w])
                    if c == OUTG[q][-1]:
                        nc.scalar.dma_start(
                            out=outS[sb * P:(sb + 1) * P,
                                     goff[q]:goff[q] + glen[q]],
                            in_=ots[sb])
                t0 += L
    nc.compile()
    return nc


_NC_CACHE = {}


def _get_nc():
    if "nc" not in _NC_CACHE:
        _NC_CACHE["nc"] = _build_kernel()
    return _NC_CACHE["nc"]


def _derive_operator(A, unknown, mask):
    """Compose the scan into (Vk, rS, known) in float64."""
    A64 = np.asarray(A, dtype=np.float64)
    deg = A64.sum(axis=1)
    M = np.eye(N, dtype=np.float64)
    for u in unknown:
        M[:, u] = M @ (A64[u] / deg[u])
    r = float(mask) * M[unknown, :].sum(axis=0)
    M[unknown, :] = 0.0
    known = np.setdiff1d(np.arange(N, dtype=np.int64), unknown)
    Vk = np.ascontiguousarray(M[known][:, unknown], dtype=np.float32)
    rS = np.ascontiguousarray(r[unknown], dtype=np.float32)
    return Vk, rS, known


def _pack_weights(Vk):
    """[768, 256] f32 -> [128, 6*256] fp8 laid out [p, kpair, d, s]."""
    V = Vk.astype(NP_FP8)
    return np.ascontiguousarray(
        V.reshape(JC, P, NS).transpose(1, 0, 2).reshape(P, JC * NS))


def _pack_x(xb):
    """[768, T] fp8 -> [128, 6*T] chunk-contiguous [p, kpair, d, l] blocks."""
    blocks = []
    t0 = 0
    for L in CHUNKS:
        blk = xb[:, t0:t0 + L].reshape(JC, P, L).transpose(1, 0, 2)
        blocks.append(blk.reshape(P, JC * L))
        t0 += L
    return np.ascontiguousarray(np.concatenate(blocks, axis=1))


def _prepare_in_maps(input, A, unknown, mask):
    x = np.asarray(input, dtype=np.float32)
    unknown = np.asarray(unknown).astype(np.int64)
    Vk, rS, known = _derive_operator(A, unknown, mask)
    VkP = _pack_weights(Vk)
    in_maps = []
    for b in range(B):
        xb = x[b].T[known].astype(NP_FP8)  # [768, T]
        in_maps.append({"xC": _pack_x(xb), "VkP": VkP})
    return in_maps, unknown, rS


def kernel(input, A, unknown, mask, _spmd_kwargs=None):
    x = np.asarray(input, dtype=np.float32)
    in_maps, unknown, rS = _prepare_in_maps(input, A, unknown, mask)

    nc = _get_nc()
    res = run_bass_kernel_spmd(nc, in_maps, core_ids=list(range(B)),
                               **(_spmd_kwargs or {}))

    out = x.copy()
    for b in range(B):
        y = res.results[b]["outS"].astype(np.float32) + rS[:, None]
        out[b][:, unknown] = y.T
    return out


# revision 24
# speedup vs baseline: 1.0547x; 1.0547x over previous
"""nn_KNN Trainium2 kernel: sequential per-node neighbor-mean scan as one GEMM.

The reference's scan is a composition of per-column linear updates, so
out = x0 @ M for a precomputable M. Folding the initial mask-fill into M
(zeroing the unknown rows -> M', bias r), known columns pass through
exactly and only the 256 unknown columns need compute:

  out[:, known]   = input[:, known]                     (host pass-through)
  out[:, unknown] = input[:, known] @ Vk + r            (r added on host)

Sharding: batch b -> core b (data parallel, no collectives).

Device-side scheme (per core):
- x and Vk are quantized to fp8-e4m3 and the GEMM runs in DoubleRow perf
  mode (256-deep contraction per pass, 3 passes for K=768), which is the
  PE-minimal schedule: the PE streams one moving column per cycle
  regardless of dtype, so time = out-columns x passes. Accumulation is
  fp32 in PSUM; output is bf16. Measured end-to-end rel err 1.18e-2
  against the fp32 reference (budget 2e-2), bit-reproducible and equal
  to the host emulation of the same quantization.
- x is packed per time-chunk as contiguous [128, 6*chunk] blocks so each
  chunk is ONE dma with 128 fat descriptors; in fp8 the whole slice is
  SBUF-resident so every chunk gets its own tile (no buffer recycling
  stalls). Chunks ramp 256->1024: the DGE round-robins across queued
  transfers, so the first chunk must be small for the first matmul to
  start early, and deep issue backlogs are avoided by loading one chunk
  ahead of use.
- Inputs are issued on the sync-engine DGE ring in consumption order
  (Vk first); output quarter-DMAs live on the scalar-engine ring; psum
  drains alternate vector/scalar engines.
"""

import sys

import numpy as np

try:
    import concourse.bass  # noqa: F401
except ImportError:  # pragma: no cover
    sys.path.insert(0, "/opt/trn_rl_repo")

import concourse.bacc as bacc_mod
import concourse.mybir as mybir
from concourse.bass_utils import run_bass_kernel_spmd
from concourse.tile import TileContext

B, T, N, NS = 8, 4096, 1024, 256
NK = N - NS
P = 128
JC = NK // P  # 6 contraction tiles
KP = JC // 2  # 3 DoubleRow contraction pairs
SB = NS // P  # 2 output-partition tiles
MMF = 512  # psum bank free size (fp32)

# time-chunk schedule: small first chunks prime the pipeline fast (the DGE
# round-robins across queued transfers, so the first chunk must be small
# for the first matmul to start early); small tail chunks shorten the
# shutdown (last output store is only 512 cols)
CHUNKS = (256, 256, 512, 1024, 1024, 512, 512)
# output groups (out-DMA granularity): chunk indices per group
OUTG = ((0, 1, 2), (3,), (4,), (5,), (6,))
# out-DMAs for groups >= this index go on the sync ring (inputs are done
# by then, and the scalar queue otherwise serializes drains vs issues)
OUT_SYNC_FROM = 2
# PE warm-up matmuls on a memset tile while waiting for the first loads:
# ramps the PE clock out of its low p-state; sized to end right when the
# first input chunk lands (~2.7 us at low-pstate spacing)
WARMUP_MM = 6

BF16 = mybir.dt.bfloat16
FP8 = mybir.dt.float8e4
NP_BF16 = mybir.dt.np(mybir.dt.bfloat16)
NP_FP8 = mybir.dt.np(mybir.dt.float8e4)


def _build_kernel():
    nc = bacc_mod.Bacc("TRN2", target_bir_lowering=False, name="knn_fp8")
    f32 = mybir.dt.float32
    xC = nc.dram_tensor("xC", [P, JC * T], FP8, kind="ExternalInput")
    VkP = nc.dram_tensor("VkP", [P, JC * NS], FP8, kind="ExternalInput")
    outS = nc.dram_tensor("outS", [NS, T], BF16, kind="ExternalOutput")

    assert sum(CHUNKS) == T
    DR = mybir.MatmulPerfMode.DoubleRow

    with TileContext(nc) as tc:
        with (
            tc.tile_pool(name="consts", bufs=1) as cpool,
            tc.tile_pool(name="xt", bufs=1) as xpool,
            tc.tile_pool(name="outp", bufs=2) as opool,
            tc.tile_pool(name="ps", bufs=2, space="PSUM") as pspool,
        ):
            vk_sb = cpool.tile([P, JC * NS], FP8, tag="vk")
            nc.sync.dma_start(out=vk_sb, in_=VkP[:, :])
            vk_v = vk_sb.rearrange("p (a d s) -> p a d s", a=KP, d=2)
            vks = [vk_v[:, a] for a in range(KP)]

            # warm-up: no DMA dependency, so these run while inputs load
            warm = cpool.tile([P, 2 * MMF], FP8, tag="warm")
            nc.vector.memset(warm, 1.0)
            wv = warm.rearrange("p (d s) -> p d s", d=2)
            wps = pspool.tile([P, MMF], f32, tag="ps0_0", name="wps")
            for _ in range(WARMUP_MM):
                nc.tensor.matmul(
                    wps, lhsT=wv[:, :, :P], rhs=wv[:, :, :],
                    start=True, stop=True,
                    perf_mode=mybir.MatmulPerfMode.DoubleRow,
                )

            offs = [sum(CHUNKS[:i]) for i in range(len(CHUNKS))]

            def load_chunk(c):
                L = CHUNKS[c]
                xall = xpool.tile([P, JC * L], FP8, tag=f"xs{c}",
                                  name=f"xs{c}")
                nc.sync.dma_start(
                    out=xall,
                    in_=xC[:, JC * offs[c]:JC * (offs[c] + L)])
                return xall.rearrange("p (a d l) -> p a d l", a=KP, d=2)

            xvs = {0: load_chunk(0)}

            goff = [offs[g[0]] for g in OUTG]
            glen = [sum(CHUNKS[c] for c in g) for g in OUTG]
            ots = {}
            t0 = 0
            for c, L in enumerate(CHUNKS):
                if c + 1 < len(CHUNKS):
                    xvs[c + 1] = load_chunk(c + 1)
                xv = xvs[c]
                q = next(qi for qi, cs in enumerate(OUTG) if c in cs)
                qoff = t0 - goff[q]
                MT = (L + MMF - 1) // MMF
                for sb in range(SB):
                    if c == OUTG[q][0]:
                        ots[sb] = opool.tile([P, glen[q]], BF16,
                                             tag=f"og{q}_{sb}",
                                             name=f"og{q}_{sb}")
                    for m in range(MT):
                        w = min(MMF, L - m * MMF)
                        ps = pspool.tile([P, MMF], f32, tag=f"ps{sb}_{m}",
                                         name=f"ps{sb}_{m}")
                        for a in range(KP):
                            nc.tensor.matmul(
                                ps[:, :w],
                                lhsT=vks[a][:, :, sb * P:(sb + 1) * P],
                                rhs=xv[:, a, :, m * MMF:m * MMF + w],
                                start=(a == 0), stop=(a == KP - 1),
                                perf_mode=DR,
                            )
                        dst = ots[sb][:, qoff + m * MMF:qoff + m * MMF + w]
                        if sb == 0:
                            nc.vector.tensor_copy(out=dst, in_=ps[:, :w])
                        else:
                            nc.scalar.copy(out=dst, in_=ps[:, :w])
                    if c == OUTG[q][-1]:
                        eng = nc.sync if q >= OUT_SYNC_FROM else nc.scalar
                        eng.dma_start(
                            out=outS[sb * P:(sb + 1) * P,
                                     goff[q]:goff[q] + glen[q]],
                            in_=ots[sb])
                t0 += L
    nc.compile()
    return nc


_NC_CACHE = {}


def _get_nc():
    if "nc" not in _NC_CACHE:
        _NC_CACHE["nc"] = _build_kernel()
    return _NC_CACHE["nc"]


def _derive_operator(A, unknown, mask):
    """Compose the scan into (Vk, rS, known) in float64."""
    A64 = np.asarray(A, dtype=np.float64)
    deg = A64.sum(axis=1)
    M = np.eye(N, dtype=np.float64)
    for u in unknown:
        M[:, u] = M @ (A64[u] / deg[u])
    r = float(mask) * M[unknown, :].sum(axis=0)
    M[unknown, :] = 0.0
    known = np.setdiff1d(np.arange(N, dtype=np.int64), unknown)
    Vk = np.ascontiguousarray(M[known][:, unknown], dtype=np.float32)
    rS = np.ascontiguousarray(r[unknown], dtype=np.float32)
    return Vk, rS, known


def _pack_weights(Vk):
    """[768, 256] f32 -> [128, 6*256] fp8 laid out [p, kpair, d, s]."""
    V = Vk.astype(NP_FP8)
    return np.ascontiguousarray(
        V.reshape(JC, P, NS).transpose(1, 0, 2).reshape(P, JC * NS))


def _pack_x(xb):
    """[768, T] fp8 -> [128, 6*T] chunk-contiguous [p, kpair, d, l] blocks."""
    blocks = []
    t0 = 0
    for L in CHUNKS:
        blk = xb[:, t0:t0 + L].reshape(JC, P, L).transpose(1, 0, 2)
        blocks.append(blk.reshape(P, JC * L))
        t0 += L
    return np.ascontiguousarray(np.concatenate(blocks, axis=1))


def _prepare_in_maps(input, A, unknown, mask):
    x = np.asarray(input, dtype=np.float32)
    unknown = np.asarray(unknown).astype(np.int64)
    Vk, rS, known = _derive_operator(A, unknown, mask)
    VkP = _pack_weights(Vk)
    in_maps = []
    for b in range(B):
        xb = x[b].T[known].astype(NP_FP8)  # [768, T]
        in_maps.append({"xC": _pack_x(xb), "VkP": VkP})
    return in_maps, unknown, rS


def kernel(input, A, unknown, mask, _spmd_kwargs=None):
    x = np.asarray(input, dtype=np.float32)
    in_maps, unknown, rS = _prepare_in_maps(input, A, unknown, mask)

    nc = _get_nc()
    res = run_bass_kernel_spmd(nc, in_maps, core_ids=list(range(B)),
                               **(_spmd_kwargs or {}))

    out = x.copy()
    for b in range(B):
        y = res.results[b]["outS"].astype(np.float32) + rS[:, None]
        out[b][:, unknown] = y.T
    return out
